# revision 47
# baseline (speedup 1.0000x reference)
"""Trainium2 Bass kernel for nn_DentalAnatomyLoss (v2).

Computes, for segmentation [B=2, C=32, D=64, H=128, W=128] fp32:
  - crown/root ratio loss (per (b,c) sums over d<32 / d>=32)
  - 3D total-variation loss (mean |diff| along w, h, d)
  - returns stack([crown_root, smoothness, total_anatomy]) fp32 [3]

Pure data-parallel over the 64 (b,c) slices, 8 per NeuronCore. Each
core reduces its 32 MiB shard to a [128, ACC] fp32 partial tensor; the
host combines partials into the 3 scalars.

Layout: d-on-partitions, 2 slices per chunk pair (cp): partition
p = s*64 + d; free = (h, w) = 16384 bf16 (fp32->bf16 cast inside the
SWDGE DMA). sum|a-b| = 2*sum(max(a,b)) - sum(a) - sum(b); the signed
sums telescope to per-plane/edge sums.

Engine assignment (HW-measured rates; DVE tensor_tensor runs 2x_1p for
bf16 but TensorScalarPtrReduce/STT only 1x; per-DVE-op overhead
~0.8us, so ops are full-cp sized except the last cp):
  VectorE: gy max via tensor_tensor(max) (2x, ~37us) + gx as fused
    scalar_tensor_tensor max+accum (1x, ~71us; broadcast dummy out --
    no scratch, no reduce stream) -> ~110us.
  TensorE: block-bidiag d-diff matmuls into PSUM + accumulating
    ones-row reduce matmuls of the gy scratch + [128,4] crown/root
    group-selector matmul -> ~82us.
  ScalarE: PSUM |dz| Abs+accum drains, tiny edge row/col sums, per-cp
    drains of the accumulating PSUM rows -> ~78us.
  DMA: ~109us/core measured floor (HBM ~358 GB/s/core) -> the target.

The LAST cp uses quarter-sized V ops streamed behind each DMA quarter
so the post-last-DMA tail is one quarter's work (~7us), not a whole
cp (~27us). gy ops read w elements past their range (emitted after
the covering DMA), so no boundary ops and no gyb telescope terms.
Measured dead ends: HWDGE cannot cast (SWDGE-only loads); routing gx
through a TensorE ones-reduce chain or halving DMA count both SLOWED
the kernel (T-stream head-of-line blocking / lost T overlap).
"""

import os

import numpy as np

B, C, D, H, W = 2, 32, 64, 128, 128
NCORES = 8
JPC = (B * C) // NCORES  # slices per core
CROWN_ROOT_W = 2.0
SMOOTH_W = 1.5
EXPECTED_RATIO = 1.2

NQ = 4  # DMA quarters per chunk pair
GRP = (3, 3, 2)  # diff-PSUM group sizes (blocks) per quarter
GY_ROW = 32  # PSUM partition row of the gy-reduce accumulation
GX_ROW = 64  # PSUM partition row of the gx-reduce accumulation (odd cps)

_PROG_CACHE: dict = {}
last_exec_time_ns = None


def _layout(ncp):
    """acc column layout for the [128, acc_cols] fp32 partial tensor."""
    ndr = len(GRP) * NQ  # diff drains per cp
    col_mx = 0  # ncp*nq: per-plane sum(max w-pairs), per quarter
    col_r = col_mx + ncp * NQ  # ncp: per-plane sum(row0 + row_{h-1})
    col_c = col_r + ncp  # ncp: per-plane sum(col0 + col_{w-1})
    col_ps = col_c + ncp  # ncp: rows 0..3 = crown/root sums, row 32 = My
    col_dz = col_ps + ncp  # ncp*ndr: PSUM |dz| drains
    acc_cols = col_dz + ncp * ndr
    return ndr, col_mx, col_r, col_c, col_ps, col_dz, acc_cols


def _build_program(jpc=JPC, d=D, h=H, w=W, repeat=1, skip=()):
    """Build the (single) SPMD Bass program run identically on all cores.

    repeat>1 wraps the whole compute in a hardware For_i loop (identical
    result, used only for wall-clock timing of the kernel body).
    skip=(...) ablates op groups for engine-time attribution.
    """
    from contextlib import ExitStack

    import concourse.tile as tile
    from concourse import bacc, mybir

    f32 = mybir.dt.float32
    bf16 = mybir.dt.bfloat16
    AO = mybir.AluOpType
    AF = mybir.ActivationFunctionType

    ncp = jpc // 2
    P = 2 * d  # partitions per chunk pair
    fsz = h * w  # free size per partition (one (h,w) plane)
    qsz = fsz // NQ
    hq = h // NQ  # h-rows per quarter
    nblk = fsz // 512  # 512-col matmul blocks per cp
    bq = nblk // NQ  # blocks per quarter
    assert sum(GRP) == bq
    ndr, col_mx, col_r, col_c, col_ps, col_dz, acc_cols = _layout(ncp)

    nc = bacc.Bacc(
        "TRN2",
        target_bir_lowering=False,
        debug=False,
        enable_asserts=False,
        num_devices=NCORES,
    )
    seg = nc.dram_tensor("seg", [jpc, d, h, w], f32, kind="ExternalInput").ap()
    aux = nc.dram_tensor("aux", [P, P + 5], bf16, kind="ExternalInput").ap()
    out = nc.dram_tensor("partials", [P, acc_cols], f32, kind="ExternalOutput").ap()

    with tile.TileContext(nc) as tc, ExitStack() as ctx:
        singles = ctx.enter_context(tc.tile_pool(name="singles", bufs=1))
        xbp = ctx.enter_context(tc.tile_pool(name="xb", bufs=3))
        gyp = ctx.enter_context(tc.tile_pool(name="gy", bufs=1))
        gyqp = ctx.enter_context(tc.tile_pool(name="gyq", bufs=2))
        gxsp = ctx.enter_context(tc.tile_pool(name="gxs", bufs=1))
        psp = ctx.enter_context(tc.tile_pool(name="ps", bufs=2, space="PSUM"))
        accp = ctx.enter_context(tc.tile_pool(name="accps", bufs=2, space="PSUM"))

        aux_sb = singles.tile([P, P + 5], bf16)
        nc.sync.dma_start(out=aux_sb, in_=aux)
        bd_ap = aux_sb[:, 0:P]  # block-bidiag d-diff lhsT
        xsel_ap = aux_sb[:, P : P + 4]  # crown/root group selector lhsT
        ones_ap = aux_sb[:, P + 4 : P + 5]  # gy-reduce lhsT

        acc = singles.tile([P, acc_cols], f32)
        nc.vector.memset(acc, 0.0)
        sdump = singles.tile([P, 512], bf16)  # ScalarE psum-drain out sink
        dummy = singles.tile([P, 1], bf16)  # ScalarE broadcast out sink
        vdummy = singles.tile([P, 1], bf16)  # VectorE broadcast out sink

        def emit_gyred(gy_t, fd, acc_ps, first, final):
            # free-axis reduce of the gy max scratch: accumulate column
            # sums into PSUM row GY_ROW across all chunks of the cp.
            nchunks = (fd + 511) // 512
            for k in range(nchunks):
                c0 = k * 512
                csz = min(512, fd - c0)
                nc.tensor.matmul(
                    acc_ps[GY_ROW : GY_ROW + 1, 0:csz],
                    ones_ap,
                    gy_t[:, c0 : c0 + csz],
                    start=(first and k == 0),
                    stop=(final and k == nchunks - 1),
                    skip_group_check=True,
                )

        def cp_body(c, last=False, pending_s=None):
            # even non-last cps: gx via 2x tensor_max into a scratch that
            # ScalarE reduces (Copy+accum); the two reduce halves are handed
            # to the NEXT cp via pending_s so they never block PSUM drains
            # while waiting on VectorE.
            conv = (not last) and c % 2 == 0 and "gx" not in skip
            xb = xbp.tile([P, fsz], bf16)
            src = seg[2 * c : 2 * c + 2].rearrange("s d h w -> (s d) (h w)")
            xb3 = xb.rearrange("p (r c2) -> p r c2", c2=w)
            acc_ps = accp.tile([P, 512], f32)
            gy_tiles = []

            def emit_gy():
                # one full-cp op: h-pairs rows 0..h-2 (amortizes the
                # ~0.8us/op DVE overhead over the largest possible FD)
                fd = fsz - w
                gy_t = gyp.tile([P, fsz - w], bf16)
                nc.vector.tensor_max(
                    gy_t[:, 0:fd], xb[:, 0:fd], xb[:, w : w + fd]
                )
                gy_tiles.append((gy_t, fd))

            def emit_gy_q(q):
                # last-cp tail shrink: quarter op; q<3 reads w elements
                # into quarter q+1 (its DMA precedes this op)
                fd = qsz if q < NQ - 1 else qsz - w
                gy_t = gyqp.tile([P, qsz], bf16)
                nc.vector.tensor_max(
                    gy_t[:, 0:fd],
                    xb[:, q * qsz : q * qsz + fd],
                    xb[:, q * qsz + w : q * qsz + w + fd],
                )
                gy_tiles.append((gy_t, fd))

            def emit_gx(q=None):
                # fused max+accum over w-pairs (1x STT on V, no reduce
                # stream; broadcast out avoids a 4MB scratch). q=None:
                # full-cp op; else one quarter (last-cp tail shrink).
                # (Routing gx via tensor_max + a TensorE ones-reduce chain
                # was measured SLOWER -- the extra T stream serializes
                # against the diff matmuls.)
                r0 = 0 if q is None else q * hq
                nr = h if q is None else hq
                colq = col_mx + NQ * c + (0 if q is None else q)
                nc.vector.scalar_tensor_tensor(
                    out=vdummy.broadcast_to((P, nr, w - 1)),
                    in0=xb3[:, r0 : r0 + nr, 1:w],
                    scalar=0.0,
                    in1=xb3[:, r0 : r0 + nr, 0 : w - 1],
                    op0=AO.bypass,
                    op1=AO.max,
                    accum_out=acc[:, colq : colq + 1],
                )

            for q in range(NQ):
                if "dma" not in skip:
                    nc.gpsimd.dma_start(
                        out=xb[:, q * qsz : (q + 1) * qsz],
                        in_=src[:, q * qsz : (q + 1) * qsz],
                    )
                elif q == 0:
                    # timing ablation: tiny write so the tile allocates
                    nc.gpsimd.dma_start(out=xb[:, 0:512], in_=src[:, 0:512])
                # --- VectorE: cps 0..ncp-2 use full-cp ops (min overhead);
                # the last cp streams quarter ops behind each DMA so the
                # post-DMA tail is one quarter, not a whole cp
                if not last:
                    if q == NQ - 1:
                        if "gy" not in skip:
                            emit_gy()
                        if not conv:
                            if "gx" not in skip:
                                emit_gx()
                        else:
                            gx_t = gxsp.tile([P, h * (w - 1)], bf16)
                            nc.vector.tensor_max(
                                gx_t.rearrange("p (r c2) -> p r c2", c2=w - 1)[
                                    :, :, :
                                ],
                                xb3[:, :, 0 : w - 1],
                                xb3[:, :, 1:w],
                            )
                            for half in range(2):
                                off = half * (h * (w - 1) // 2)
                                colq = col_mx + NQ * c + 1 + half

                                def s_red(gx_t=gx_t, off=off, colq=colq):
                                    nc.scalar.activation(
                                        out=dummy.broadcast_to(
                                            (P, h * (w - 1) // 2)
                                        ),
                                        in_=gx_t[:, off : off + h * (w - 1) // 2],
                                        func=AF.Copy,
                                        accum_out=acc[:, colq : colq + 1],
                                    )

                                pending_s.append(s_red)
                else:
                    if "gx" not in skip:
                        emit_gx(q)
                    if "gy" not in skip and q > 0:
                        emit_gy_q(q - 1)
                        if q == NQ - 1:
                            emit_gy_q(q)
                # --- TensorE: d-diff + group-sum matmuls; ScalarE drains ---
                for g, gsz in enumerate(GRP):
                    goff = q * bq + sum(GRP[:g])
                    if "gz" not in skip:
                        # constant-size PSUM tile (uniform pool slots); the
                        # last group only uses gsz of the GRP[0] blocks
                        ps = psp.tile([P, GRP[0], 512], f32)
                        for j in range(gsz):
                            blk = goff + j
                            nc.tensor.matmul(
                                ps[:, j, :],
                                bd_ap,
                                xb[:, blk * 512 : (blk + 1) * 512],
                                start=True,
                                stop=True,
                            )
                    if "xs" not in skip:
                        for j in range(gsz):
                            blk = goff + j
                            nc.tensor.matmul(
                                acc_ps[0:4, :],
                                xsel_ap,
                                xb[:, blk * 512 : (blk + 1) * 512],
                                start=(blk == 0),
                                stop=(blk == nblk - 1),
                                skip_group_check=True,
                            )
                    if "gz" not in skip and "drain" not in skip:
                        colx = col_dz + ndr * c + len(GRP) * q + g
                        nc.scalar.activation(
                            out=dummy.broadcast_to((P, gsz, 512)),
                            in_=ps[:, 0:gsz, :],
                            func=AF.Abs,
                            accum_out=acc[:, colx : colx + 1],
                        )
                    # previous cp's deferred gx scratch reduces (ScalarE)
                    if pending_s and q in (1, 2) and g == len(GRP) - 1:
                        pending_s.pop(0)()
                # --- TensorE: gy scratch reduce (after the V max ops) ---
                if "gy" not in skip and "gyred" not in skip:
                    if not last:
                        if q == NQ - 1:
                            emit_gyred(*gy_tiles[0], acc_ps, True, True)
                    elif q > 0:
                        emit_gyred(*gy_tiles[q - 1], acc_ps, q == 1, False)
                        if q == NQ - 1:
                            emit_gyred(*gy_tiles[q], acc_ps, False, True)

            # --- ScalarE: edge sums + accumulating-PSUM drains ---
            if "edges" not in skip:
                rows = xb3[:, 0 : h : h - 1, :]
                nc.scalar.activation(
                    out=dummy.broadcast_to((P, 2, w)),
                    in_=rows,
                    func=AF.Copy,
                    accum_out=acc[:, col_r + c : col_r + c + 1],
                )
                colsv = xb.rearrange("p (r c2) -> p c2 r", c2=w)[:, 0 : w : w - 1, :]
                nc.scalar.activation(
                    out=dummy.broadcast_to((P, 2, h)),
                    in_=colsv,
                    func=AF.Copy,
                    accum_out=acc[:, col_c + c : col_c + c + 1],
                )
            if "xs" not in skip:
                nc.scalar.activation(
                    out=sdump[0:4, 0:512],
                    in_=acc_ps[0:4, :],
                    func=AF.Copy,
                    accum_out=acc[0:4, col_ps + c : col_ps + c + 1],
                )
            if "gy" not in skip and "gyred" not in skip:
                nc.scalar.activation(
                    out=sdump[GY_ROW : GY_ROW + 1, 0:512],
                    in_=acc_ps[GY_ROW : GY_ROW + 1, :],
                    func=AF.Copy,
                    accum_out=acc[GY_ROW : GY_ROW + 1, col_ps + c : col_ps + c + 1],
                )


        def all_cps():
            pending_s = []
            for c in range(ncp):
                cp_body(c, last=(c == ncp - 1), pending_s=pending_s)
            assert not pending_s, "unconsumed deferred ScalarE reduces"

        if repeat == 1:
            all_cps()
        else:
            with tc.For_i(0, repeat, 1):
                all_cps()
        nc.sync.dma_start(out=out, in_=acc)

    nc.compile()
    return nc


def _get_program():
    key = "full"
    if key not in _PROG_CACHE:
        _PROG_CACHE[key] = _build_program()
    return _PROG_CACHE[key]


def _aux_np(d=D):
    """[2d, 2d+5] bf16 lhsT bundle: cols 0..2d-1 block-bidiag (out row m =
    x[m+1]-x[m] within each slice; cols d-1, 2d-1 zero), cols 2d..2d+3
    crown/root group selectors, col 2d+4 ones (gy reduce)."""
    import ml_dtypes

    P = 2 * d
    a = np.zeros((P, P + 5), dtype=np.float32)
    for col in range(P - 1):
        if col == d - 1:
            continue
        a[col, col] = -1.0
        a[col + 1, col] = 1.0
    hd = d // 2
    for j in range(4):
        a[j * hd : (j + 1) * hd, P + j] = 1.0
    a[:, P + 4] = 1.0
    return a.astype(ml_dtypes.bfloat16)


def _combine(partials, jpc=JPC, d=D, h=H, w=W):
    """Host-side finish: per-core [2d, acc_cols] fp32 partials -> [3]."""
    ncp = jpc // 2
    ndr, col_mx, col_r, col_c, col_ps, col_dz, acc_cols = _layout(ncp)

    nslice = jpc * len(partials)
    crown = np.zeros(nslice, dtype=np.float64)
    root = np.zeros(nslice, dtype=np.float64)
    gxy_sum = 0.0
    gz_sum = 0.0
    for k, p in enumerate(partials):
        p = p.astype(np.float64)
        for c in range(ncp):
            cr0, rt0, cr1, rt1 = p[0:4, col_ps + c]
            my = p[GY_ROW, col_ps + c]
            s_cp = cr0 + rt0 + cr1 + rt1
            r_cp = p[:, col_r + c].sum()
            c_cp = p[:, col_c + c].sum()
            mx = p[:, col_mx + NQ * c : col_mx + NQ * c + NQ].sum()
            # sum|a-b| = 2*sum(max) - sum(a) - sum(b); signed sums telescope
            gxy_sum += 2.0 * my - 2.0 * s_cp + r_cp
            gxy_sum += 2.0 * mx - 2.0 * s_cp + c_cp
            sl = k * jpc + 2 * c
            crown[sl], root[sl] = cr0, rt0
            crown[sl + 1], root[sl + 1] = cr1, rt1
        # diff rows d-1 and 2d-1 are |0| = 0 (zeroed bidiag columns)
        gz_sum += p[:, col_dz : col_dz + ncp * ndr].sum()

    total = crown + root
    valid = (total > 0) & (root > 0)
    safe_root = np.where(root > 0, root, 1.0)
    ratio_loss = np.where(valid, (crown / safe_root - EXPECTED_RATIO) ** 2, 0.0)
    cr_loss = ratio_loss.sum() / nslice

    nxy = nslice * d * h * (w - 1)  # == nslice * d * (h-1) * w
    nz = nslice * (d - 1) * h * w
    tv = gxy_sum / nxy + gz_sum / nz

    crown_root = cr_loss * CROWN_ROOT_W
    smoothness = tv * SMOOTH_W
    return np.array(
        [crown_root, smoothness, crown_root + smoothness], dtype=np.float32
    )


def kernel(segmentation: np.ndarray) -> np.ndarray:
    global last_exec_time_ns
    from concourse.bass_utils import run_bass_kernel_spmd

    seg = np.ascontiguousarray(np.asarray(segmentation), dtype=np.float32)
    assert seg.shape == (B, C, D, H, W)
    nc = _get_program()

    aux = _aux_np()
    shards = seg.reshape(B * C, D, H, W)
    in_maps = [
        {"seg": np.ascontiguousarray(shards[k * JPC : (k + 1) * JPC]), "aux": aux}
        for k in range(NCORES)
    ]
    trace = bool(os.environ.get("BASS_TRACE"))
    res = run_bass_kernel_spmd(nc, in_maps, list(range(NCORES)), trace=trace)
    last_exec_time_ns = res.exec_time_ns
    partials = [res.results[k]["partials"] for k in range(NCORES)]
    return _combine(partials)
